# revision 7
# baseline (speedup 1.0000x reference)
"""Causal multi-head self-attention (RoPE) Trainium2 Bass kernel.

Contract: kernel(**inputs) takes the FULL unsharded inputs
  x [B=2, S=2048, D=1024] f32, qkv_w [3072, 1024] f32,
  out_w [1024, 1024] f32, token_positions [2048] i32
and returns the FULL output [2, 2048, 1024] f32.

Sharding: B (2) x head-groups (4 heads each) -> 8 cores.
Core c: batch c//4, heads 4*(c%4) .. 4*(c%4)+3.
Each core computes a partial output projection over its 256 local
head-dims; the host sums the 4 partials per batch.

Device-side layout is fully transposed (partition = feature dim):
  - qkv projection emits q', k' in [d_k, S] layout and v in [S, d_k].
  - RoPE is applied as q' = cos (.) q + sin (.) qJ where qJ = J q comes
    from a host-premultiplied signed-permutation of the q weights
    (rotate-half trick), so RoPE is 3 elementwise ops, no strided pairs.
  - scores are computed k-major (scores^T [sk, sq]); softmax skips the
    max subtraction (scores are bounded ~|4.5| for this distribution;
    exp stays in [e-5, e5]) so no cross-partition max is needed.
  - attn @ v appends a ones-column to v so the softmax denominator
    falls out of the same matmul (row 64 of the psum).
  - causal masking: diagonal tiles use persistent pre-zeroed exp tiles
    plus one [128,128] triangular multiplicative mask.
"""

import os
import sys

import numpy as np

_REPO_CANDIDATES = [
    "/opt/trn_rl_repo",
    "/root/.axon_site/_ro/trn_rl_repo",
]


def _ensure_repo_on_path():
    try:
        import concourse.bass  # noqa: F401
        return
    except ImportError:
        pass
    for p in _REPO_CANDIDATES:
        if os.path.isdir(p) and p not in sys.path:
            sys.path.insert(0, p)
    import concourse.bass  # noqa: F401


NUM_HEADS = 16
ROPE_THETA = 10000.0
D = 1024
DK = 64
H_LOC = 4          # heads per core
N_CORES = 8


# --------------------------------------------------------------------------
# Device program
# --------------------------------------------------------------------------

def build_nc(S=2048):
    """Build the per-core Bass program (SPMD, same on all 8 cores)."""
    _ensure_repo_on_path()
    import concourse.mybir as mybir
    from concourse import bacc
    from concourse.tile import TileContext
    from concourse.alu_op_type import AluOpType

    dt = mybir.dt
    f32, f32r = dt.float32, dt.float32r
    Exp = mybir.ActivationFunctionType.Exp
    MUL, ADD = AluOpType.mult, AluOpType.add

    NC = S // 512    # 512-wide s-chunks
    NT = S // 128    # 128-wide s-tiles
    KD = D // 128    # d-chunks (contraction)

    nc = bacc.Bacc(None, target_bir_lowering=False, debug=False)

    xT = nc.dram_tensor("xT", [D, S], f32, kind="ExternalInput")
    wqkT = nc.dram_tensor("wqkT", [D, 1024], f32, kind="ExternalInput")
    wvT = nc.dram_tensor("wvT", [D, 256], f32, kind="ExternalInput")
    woT = nc.dram_tensor("woT", [256, 1024], f32, kind="ExternalInput")
    cosT = nc.dram_tensor("cosT", [128, S], f32, kind="ExternalInput")
    sinT = nc.dram_tensor("sinT", [128, S], f32, kind="ExternalInput")
    tri = nc.dram_tensor("tri", [128, 128], f32, kind="ExternalInput")
    consts = nc.dram_tensor("consts", [128, 448], f32, kind="ExternalInput")
    oT = nc.dram_tensor("oT", [D, S], f32, kind="ExternalOutput")

    r = lambda ap: ap.bitcast(f32r)

    with TileContext(nc) as tc, \
         nc.allow_low_precision(reason="float32r is bit-compatible with float32"):
        with tc.tile_pool(name="persist", bufs=1) as P:
            qp = [P.tile([128, S], f32r, name=f"qp{p}") for p in range(2)]
            kp = [P.tile([128, S], f32r, name=f"kp{p}") for p in range(2)]
            vbig = P.tile([128, 260 * NT], f32r, name="vbig")
            wo_sb = [P.tile([128, 1024], f32r, name=f"wo{i}") for i in range(2)]
            trit = P.tile([128, 128], f32, name="trit")
            ones_row = P.tile([1, 64], f32r, name="ones_row")

            nc.sync.dma_start(out=trit[:], in_=tri[:])
            nc.sync.dma_start(out=ones_row[:], in_=r(consts[0:1, 0:64]))
            for i in range(2):
                nc.sync.dma_start(out=wo_sb[i][:], in_=r(woT[128 * i:128 * (i + 1), :]))
            # ones columns of v_ext (col 64 of each head's 65-col block)
            ones_cols = vbig[:].rearrange(
                "p (st h j) -> p st h j", st=NT, h=H_LOC)[:, :, :, 64:65]
            ones_src = r(consts[:, 0:NT * H_LOC]).rearrange(
                "p (st h one) -> p st h one", h=H_LOC, one=1)
            nc.sync.dma_start(out=ones_cols, in_=ones_src)

            # ---------------- projection phase ----------------
            with tc.tile_pool(name="proj", bufs=1) as PP:
                cos_sb = PP.tile([128, S], f32, name="cos_sb")
                sin_sb = PP.tile([128, S], f32, name="sin_sb")
                nc.sync.dma_start(out=cos_sb[:], in_=cosT[:])
                nc.sync.dma_start(out=sin_sb[:], in_=sinT[:])
                xt_sb = []
                for t in range(KD):
                    xt = PP.tile([128, S], f32r, name=f"xt{t}")
                    nc.sync.dma_start(out=xt[:], in_=r(xT[128 * t:128 * (t + 1), :]))
                    xt_sb.append(xt)
                wq_sb = []
                for t in range(KD):
                    w = PP.tile([128, 1024], f32r, name=f"wq{t}")
                    nc.sync.dma_start(out=w[:], in_=r(wqkT[128 * t:128 * (t + 1), :]))
                    wq_sb.append(w)

                with tc.tile_pool(name="ps_proj", bufs=1, space="PSUM") as PSP, \
                     tc.tile_pool(name="rtmp", bufs=1) as RT:
                    for j in range(NC):
                        sj = slice(512 * j, 512 * (j + 1))
                        for grp in range(2):        # 0: q,qJ   1: k,kJ
                            pst = [PSP.tile([128, 512], f32, tag=f"m{grp * 4 + mi}",
                                            name=f"ps_m{grp * 4 + mi}_{j}")
                                   for mi in range(4)]
                            for t in range(KD):
                                for mi in range(4):
                                    m = grp * 4 + mi
                                    nc.tensor.matmul(
                                        pst[mi][:],
                                        wq_sb[t][:, 128 * m:128 * (m + 1)],
                                        xt_sb[t][:, sj],
                                        start=(t == 0), stop=(t == KD - 1))
                            dst = qp if grp == 0 else kp
                            for pi in range(2):
                                t1 = RT.tile([128, 512], f32, tag="r1", bufs=2)
                                t2 = RT.tile([128, 512], f32, tag="r2", bufs=2)
                                nc.vector.tensor_tensor(t1[:], pst[pi][:], cos_sb[:, sj], MUL)
                                nc.vector.tensor_tensor(t2[:], pst[2 + pi][:], sin_sb[:, sj], MUL)
                                nc.vector.tensor_tensor(dst[pi][:, sj], t1[:], t2[:], ADD)

                # ---------------- v projection ----------------
                with tc.tile_pool(name="ps_v", bufs=1, space="PSUM") as PV, \
                     tc.tile_pool(name="wv_pool", bufs=1) as WV:
                    wv_sb = []
                    for t in range(KD):
                        w = WV.tile([128, 256], f32r, name=f"wv{t}")
                        nc.sync.dma_start(out=w[:], in_=r(wvT[128 * t:128 * (t + 1), :]))
                        wv_sb.append(w)
                    for sg in range((NT + 7) // 8):
                        gsz = min(8, NT - sg * 8)
                        pvs = [PV.tile([128, 256], f32, tag=f"v{k}", name=f"ps_v{k}_{sg}") for k in range(gsz)]
                        for t in range(KD):
                            for k in range(gsz):
                                st = sg * 8 + k
                                nc.tensor.matmul(
                                    pvs[k][:],
                                    xt_sb[t][:, 128 * st:128 * (st + 1)],
                                    wv_sb[t][:],
                                    start=(t == 0), stop=(t == KD - 1))
                        for k in range(gsz):
                            st = sg * 8 + k
                            dstv = vbig[:, 260 * st:260 * (st + 1)].rearrange(
                                "p (h j) -> p h j", j=65)[:, :, 0:64]
                            srcv = pvs[k][:].rearrange("p (h j) -> p h j", j=64)
                            nc.vector.tensor_copy(dstv, srcv)

            # ---------------- attention phase ----------------
            with tc.tile_pool(name="attn", bufs=1) as AT:
                ao = [AT.tile([128, S], f32r, name=f"ao{p}") for p in range(2)]
                diag_et = [[AT.tile([128, 512], f32r, name=f"diag{hh}_{di}")
                            for di in range(4)] for hh in range(2)]
                for hh in range(2):
                    for di in range(1, 4):
                        nc.sync.dma_start(out=diag_et[hh][di][:, 0:128 * di],
                                          in_=r(consts[:, 64:64 + 128 * di]))

                with tc.tile_pool(name="ps_att", bufs=1, space="PSUM") as PSA, \
                     tc.tile_pool(name="et_pool", bufs=1) as ET, \
                     tc.tile_pool(name="nrm_pool", bufs=1) as NP:
                    for p in range(2):
                        for j in range(NC):
                            sj = slice(512 * j, 512 * (j + 1))
                            po = [PSA.tile([65, 512], f32, tag=f"o{hh}", name=f"ps_o{hh}_{p}_{j}")
                                  for hh in range(2)]
                            n_i = 4 * j + 4
                            for i in range(n_i):
                                di = i - 4 * j
                                ets = []
                                for hh in range(2):
                                    hs = slice(64 * hh, 64 * (hh + 1))
                                    if di < 0:
                                        ps = PSA.tile([128, 512], f32,
                                                      tag=f"s{hh}", bufs=2)
                                        nc.tensor.matmul(
                                            ps[:],
                                            kp[p][hs, 128 * i:128 * (i + 1)],
                                            qp[p][hs, sj],
                                            start=True, stop=True)
                                        et = ET.tile([128, 512], f32r,
                                                     tag=f"et{hh}", bufs=3)
                                        nc.scalar.activation(et[:], ps[:], Exp)
                                    else:
                                        w0 = 128 * di
                                        n_w = 512 - w0
                                        ps = PSA.tile([128, n_w], f32,
                                                      tag=f"s{hh}", bufs=2)
                                        nc.tensor.matmul(
                                            ps[:],
                                            kp[p][hs, 128 * i:128 * (i + 1)],
                                            qp[p][hs, 512 * j + w0:512 * (j + 1)],
                                            start=True, stop=True)
                                        et = diag_et[hh][di]
                                        nc.scalar.activation(et[:, w0:512], ps[:], Exp)
                                        nc.vector.tensor_tensor(
                                            et[:, w0:w0 + 128], et[:, w0:w0 + 128],
                                            trit[:], MUL)
                                    ets.append(et)
                                for hh in range(2):
                                    h = 2 * p + hh
                                    vsl = vbig[:, 260 * i + 65 * h:260 * i + 65 * (h + 1)]
                                    nc.tensor.matmul(
                                        po[hh][:], vsl, ets[hh][:],
                                        start=(i == 0), stop=(i == n_i - 1))
                            # normalize
                            for hh in range(2):
                                rc = NP.tile([1, 512], f32r, tag="rc", bufs=2)
                                nc.vector.reciprocal(rc[:], po[hh][64:65, :])
                                pb = PSA.tile([64, 512], f32, tag="pb", bufs=2)
                                nc.tensor.matmul(pb[:], ones_row[:], rc[:],
                                                 start=True, stop=True)
                                bs = NP.tile([64, 512], f32, tag="bs", bufs=2)
                                nc.vector.tensor_copy(bs[:], pb[:])
                                nc.vector.tensor_tensor(
                                    ao[p][64 * hh:64 * (hh + 1), sj],
                                    po[hh][0:64, :], bs[:], MUL)

                # ---------------- output projection ----------------
                with tc.tile_pool(name="ps_f", bufs=1, space="PSUM") as PF, \
                     tc.tile_pool(name="ostage", bufs=1) as OS:
                    for j in range(NC):
                        sj = slice(512 * j, 512 * (j + 1))
                        for e in range(8):
                            pf = PF.tile([128, 512], f32, tag="pf", bufs=4)
                            for kc in range(2):
                                nc.tensor.matmul(
                                    pf[:],
                                    wo_sb[kc][:, 128 * e:128 * (e + 1)],
                                    ao[kc][:, sj],
                                    start=(kc == 0), stop=(kc == 1))
                            ot = OS.tile([128, 512], f32, tag="ot", bufs=4)
                            nc.any.tensor_copy(ot[:], pf[:])
                            nc.sync.dma_start(
                                out=oT[128 * e:128 * (e + 1), sj], in_=ot[:])

    nc.finalize()
    return nc


# --------------------------------------------------------------------------
# Host-side input prep / output assembly
# --------------------------------------------------------------------------

def prep_core_inputs(x, qkv_w, out_w, token_positions, S=2048):
    """Build the 8 per-core input maps (numpy, host-side sharding)."""
    x = np.asarray(x, dtype=np.float32)
    qkv_w = np.asarray(qkv_w, dtype=np.float32)
    out_w = np.asarray(out_w, dtype=np.float32)
    pos = np.asarray(token_positions).astype(np.float32)

    B = x.shape[0]
    inv_freq = 1.0 / (ROPE_THETA ** (np.arange(0, DK, 2, dtype=np.float32) / DK))
    ang = pos[:, None] * inv_freq[None, :]          # [S, 32]
    cos32 = np.cos(ang).astype(np.float32)          # [S, 32]
    sin32 = np.sin(ang).astype(np.float32)
    # rows: dk index (interleaved pairs duplicated), repeated for 2 heads
    cosT = np.repeat(cos32.T, 2, axis=0)            # [64, S]
    sinT = np.repeat(sin32.T, 2, axis=0)
    cosT = np.ascontiguousarray(np.tile(cosT, (2, 1)))  # [128, S]
    sinT = np.ascontiguousarray(np.tile(sinT, (2, 1)))

    tri = np.ascontiguousarray(
        (np.arange(128)[None, :] >= np.arange(128)[:, None]).astype(np.float32))
    consts_arr = np.zeros((128, 448), dtype=np.float32)
    consts_arr[:, 0:64] = 1.0

    xT = [np.ascontiguousarray(x[b].T) for b in range(B)]   # [D, S]

    scale = 1.0 / np.sqrt(np.float32(DK))

    def jmat(w):
        """Rotate-half premultiply: (J w)[2k] = -w[2k+1]; (J w)[2k+1] = w[2k]."""
        out = np.empty_like(w)
        out[0::2] = -w[1::2]
        out[1::2] = w[0::2]
        return out

    in_maps = []
    for c in range(N_CORES):
        b = c // 4
        g = c % 4
        hsl = slice(64 * H_LOC * g, 64 * H_LOC * (g + 1))     # 256 dims
        wq = qkv_w[0 * D:1 * D][hsl] * scale                  # [256, 1024]
        wk = qkv_w[1 * D:2 * D][hsl]
        wv = qkv_w[2 * D:3 * D][hsl]
        wqkj = np.concatenate([wq, jmat(wq), wk, jmat(wk)], axis=0)  # [1024, 1024]
        in_maps.append({
            "xT": xT[b],
            "wqkT": np.ascontiguousarray(wqkj.T),
            "wvT": np.ascontiguousarray(wv.T),
            "woT": np.ascontiguousarray(out_w[:, hsl].T),     # [256, 1024]
            "cosT": cosT,
            "consts": consts_arr,
            "sinT": sinT,
            "tri": tri,
        })
    return in_maps


def assemble_output(results, B=2, S=2048):
    """Sum per-core partial oT [D, S] over each batch's 4 cores, transpose."""
    out = np.empty((B, S, D), dtype=np.float32)
    for b in range(B):
        acc = results[4 * b]["oT"].astype(np.float32).copy()
        for g in range(1, 4):
            acc += results[4 * b + g]["oT"]
        out[b] = acc.T
    return out


_NC_CACHE = {}


def get_nc(S=2048):
    if S not in _NC_CACHE:
        _NC_CACHE[S] = build_nc(S)
    return _NC_CACHE[S]


def kernel(x, qkv_w, out_w, token_positions):
    _ensure_repo_on_path()
    from concourse.bass_utils import run_bass_kernel_spmd

    x = np.asarray(x)
    S = x.shape[1]
    in_maps = prep_core_inputs(x, qkv_w, out_w, token_positions, S=S)
    nc = get_nc(S)
    res = run_bass_kernel_spmd(nc, in_maps, core_ids=list(range(N_CORES)))
    return assemble_output(res.results, B=x.shape[0], S=S)
